# revision 1
# baseline (speedup 1.0000x reference)
"""Trainium2 Bass kernel for multi-head self-attention (B=2, N=4096, C=512, H=8).

Sharding: 8 cores = 2 batches x 4 head-pairs. Core c handles batch c//4 and
heads {2*(c%4), 2*(c%4)+1}. Each core computes its two heads' attention over
all 4096 tokens and a partial output projection restricted to its heads' 128
channels; the host sums the 4 partials per batch (the tensor-parallel proj
all-reduce) and adds b_proj.

Dataflow (fp16 operands, fp32 PSUM accumulation, scores never in DRAM):
  xT arrives host-pre-transposed [512, 4096] fp16 -> SBUF (no PE transposes)
  kT/qT = (w^T @ xT)              [128, 4096]  (rows 0-63 head0, 64-127 head1)
  v natural = xT-block^T @ wv     [4096, 130]  per tile: [Vh0 | 1 | Vh1 | 1]
  per 512-query chunk, per 128-key tile:
    S^T = kslc^T @ qT   (two row-packed K=64 matmuls -> PSUM [128, 1024])
    P^T = exp(SCALE * S^T) fp16   (ScalarE, straight out of PSUM; a fraction
                                   of steps use a custom DVE exp2 instead to
                                   share the exp load with the vector engine)
    PV += [V|1]^T @ P^T           (PSUM accumulate; row 64 = denominators)
  chunk epilogue: raw PV drain (frees PSUM fast, hidden under deferred S/exp
    of the next chunk), then recip/broadcast/scale off the critical path and
    ONE packed proj matmul per 128-query tile (both heads in one K=128).
"""

import os
import sys

if "/opt/trn_rl_repo" not in sys.path:
    sys.path.insert(0, "/opt/trn_rl_repo")

import numpy as np

import concourse.mybir as mybir
import concourse.tile as tile
from concourse import bacc

B, N, C, H = 2, 4096, 512, 8
D = C // H
SCALE = D**-0.5
F32 = mybir.dt.float32
F16 = mybir.dt.float16
I16 = mybir.dt.int16
MUL = mybir.AluOpType.mult
EXP = mybir.ActivationFunctionType.Exp

MM_DT_NAME = "f16"  # informational (test.py prints it)

# every DVE_EXP_EVERY-th key-tile step computes exp on the vector engine via
# a custom DVE op (fp16-Schraudolph: bits of 2^y assembled in fp arithmetic,
# written through an int16 bitcast). 0 disables.
DVE_EXP_EVERY = int(os.environ.get("ATTN_DVE_EXP_EVERY", "0"))

_EXP_OP = None


def _get_exp_op():
    """Register (once) a custom DVE op computing fp16 exp bit patterns.

    Schraudolph in fp16: bits = relu(x*C0 + C1) with C0 = SCALE*log2e*1024,
    C1 = 15360 - 1024*sigma (sigma = 0.0579 makes the sawtooth mean-zero).
    Rounded to int16 at writeback, the result IS the fp16 encoding of
    ~e^(x*SCALE) (max rel err ~4%, zero-mean; the softmax ratio washes it
    out). relu clamps the x << 0 tail to +0.0.
    """
    global _EXP_OP
    if _EXP_OP is not None:
        return _EXP_OP
    from concourse import dve_ops
    from concourse.dve_spec import Spec, Src0, C0, C1, relu, lower
    from concourse.dve_uop import DveOpSpec

    name = "EXP2F16_ANT"
    spec = Spec(body=relu(Src0 * C0 + C1))
    existing = [op for op in dve_ops.OPS if op.name == name]
    if existing:
        _EXP_OP = existing[0]
        return _EXP_OP
    opcode = dve_ops._CUSTOM_DVE_ROW_BASE + len(dve_ops.OPS)
    uops = lower(spec, ver="v3")
    ds = DveOpSpec(name=name, opcode=opcode, uops=uops, rd1_en=False)
    sha = ds.sha("v3")
    op = dve_ops.DveOp(name, spec, subdim=False, uops_sha={"v3": sha})
    dve_ops.OPS.append(op)
    dve_ops.CUSTOM_DVE_SPECS[name] = spec
    dve_ops._SUB_OPCODE_FOR_NAME[name] = opcode
    _EXP_OP = op
    return op


def _exp_consts():
    """(C0, C1) for the custom exp op."""
    import math

    c0 = SCALE * math.log2(math.e) * 1024.0
    c1 = 15360.0 - 1024.0 * 0.0579
    return c0, c1


def build(tokens=N):
    T = tokens
    n_xt = T // 128  # key tiles
    n_s = T // 512  # token slices for kT/qT production
    n_qc = T // 512  # query chunks

    use_dve_exp = DVE_EXP_EVERY > 0
    if use_dve_exp:
        exp_op = _get_exp_op()
        ec0, ec1 = _exp_consts()

    nc = bacc.Bacc(None)
    xt = nc.dram_tensor("xt", [C, T], F16, kind="ExternalInput")  # x[b].T
    out = nc.dram_tensor("out", [T, C], F32, kind="ExternalOutput")
    # concatenated [wq | wk | wv | wp], each [128, 512] host-prepped fp16
    # (wq/wk/wv: w_[p, kc*128 + j] = w[kc*128 + p, j]; wp natural rows)
    wall = nc.dram_tensor("wall", [128, 2048], F16, kind="ExternalInput")

    with tile.TileContext(nc) as tc:
        with tc.tile_pool(name="persist", bufs=1) as pp:
            w_all = pp.tile([128, 2048], F16, tag="w_all", name="w_all")
            nc.sync.dma_start(out=w_all[:], in_=wall[:, :])
            wq_sb = w_all[:, 0:512]
            wk_sb = w_all[:, 512:1024]
            wv_sb = w_all[:, 1024:1536]
            wp_sb = w_all[:, 1536:2048]
            # ones row for broadcasting recip rows across 64 partitions
            ones1 = pp.tile([1, 64], F16, tag="ones1")
            nc.gpsimd.memset(ones1[:], 1.0)
            # warm the Exp activation table (~1.3us) during the input DMAs
            # instead of on the first real exp
            dum = pp.tile([1, 1], F32, tag="dum")
            nc.gpsimd.memset(dum[:], 0.0)
            nc.scalar.activation(dum[:], dum[:], EXP, scale=1.0)

            # all 4 c-chunks of xT in one flat tile: chunk kc at cols [kc*T..)
            xall = pp.tile([128, 4 * T], F16, tag="xall", name="xall")

            def xslc(kc, sl):
                return xall[:, kc * T + sl.start : kc * T + sl.stop]
            kT = [
                pp.tile([128, 512], F16, tag=f"kT{s}", name=f"kT{s}")
                for s in range(n_s)
            ]
            qT = [
                pp.tile([128, 512], F16, tag=f"qT{s}", name=f"qT{s}")
                for s in range(n_s)
            ]
            v = [
                pp.tile([128, 130], F16, tag=f"v{t}", name=f"v{t}")
                for t in range(n_xt)
            ]

            def attn_S(qc, kt, psS, ptp, on_dve=False):
                """Scores + exp for one (chunk, key-tile); returns pt tile."""
                sc = psS.tile([128, 1024], F32, tag="sc", name="sc")
                kslc = kT[kt // 4][:, (kt % 4) * 128 : (kt % 4 + 1) * 128]
                nc.tensor.matmul(
                    sc[:, 0:512],
                    kslc[0:64, :],
                    qT[qc][0:64, :],
                    start=True,
                    stop=True,
                    tile_position=(0, 0),
                )
                nc.tensor.matmul(
                    sc[:, 512:1024],
                    kslc[64:128, :],
                    qT[qc][64:128, :],
                    start=True,
                    stop=True,
                    tile_position=(64, 0),
                )
                pt = ptp.tile([128, 1024], F16, tag="pt", name="pt")
                if on_dve:
                    nc.vector._custom_dve(
                        exp_op,
                        out=pt[:].bitcast(I16),
                        in0=sc[:],
                        s0=ec0,
                        s1=ec1,
                    )
                else:
                    nc.scalar.activation(pt[:], sc[:], EXP, scale=SCALE)
                return pt

            def attn_PV(kt, pt, pv0, pv1, start=None, stop=None):
                start = (kt == 0) if start is None else start
                stop = (kt == n_xt - 1) if stop is None else stop
                nc.tensor.matmul(
                    pv0[:], v[kt][:, 0:65], pt[:, 0:512], start=start, stop=stop
                )
                nc.tensor.matmul(
                    pv1[:], v[kt][:, 65:130], pt[:, 512:1024], start=start, stop=stop
                )

            CPY = mybir.ActivationFunctionType.Copy

            def epi_drain(pv0, pv1, smp, otp, final=False):
                """Fast PV-bank release: raw copies only (DVE, ~2.7us). In the
                final epilogue ScalarE is idle (no more exps) and `copy` lives
                in the exp activation table, so the praw halves go there to
                halve the serial drain chain."""
                dna = smp.tile([1, 512], F32, tag="dna", name="dna")
                dnb = smp.tile([1, 512], F32, tag="dnb", name="dnb")
                nc.vector.tensor_copy(dna[:], pv0[64:65, :])
                nc.vector.tensor_copy(dnb[:], pv1[64:65, :])
                praw = otp.tile([128, 512], F32, tag="praw", name="praw")
                if final:
                    nc.scalar.activation(praw[0:64, :], pv0[0:64, :], CPY)
                    nc.scalar.activation(praw[64:128, :], pv1[0:64, :], CPY)
                else:
                    nc.vector.tensor_copy(praw[0:64, :], pv0[0:64, :])
                    nc.vector.tensor_copy(praw[64:128, :], pv1[0:64, :])
                return dna, dnb, praw

            def epi_scale(dna, dnb, praw, smp, otp, psB):
                """Off-critical-path: recip, broadcast matmuls, fused scale."""
                rca = smp.tile([1, 512], F32, tag="rca", name="rca")
                rcb = smp.tile([1, 512], F32, tag="rcb", name="rcb")
                nc.vector.reciprocal_approx_fast(rca[:], dna[:])
                nc.vector.reciprocal_approx_fast(rcb[:], dnb[:])
                rha = smp.tile([1, 512], F16, tag="rha", name="rha")
                rhb = smp.tile([1, 512], F16, tag="rhb", name="rhb")
                nc.vector.tensor_copy(rha[:], rca[:])
                nc.vector.tensor_copy(rhb[:], rcb[:])
                bc = psB.tile([128, 512], F32, tag="pb", name="bc")
                nc.tensor.matmul(bc[0:64, :], ones1[:], rha[:], start=True, stop=True)
                nc.tensor.matmul(
                    bc[64:128, :],
                    ones1[:],
                    rhb[:],
                    start=True,
                    stop=True,
                    tile_position=(0, 64),
                )
                outT = otp.tile([128, 512], F16, tag="outT", name="outT")
                nc.vector.tensor_tensor(outT[:], praw[:], bc[:], MUL)
                return outT

            def proj_qtile(qc, qs, outT, psP, obp, final=False):
                i = qc * 4 + qs
                pj = psP.tile([128, 512], F32, tag="pb", name="pj")
                nc.tensor.matmul(
                    pj[:],
                    outT[:, qs * 128 : (qs + 1) * 128],
                    wp_sb[:],
                    start=True,
                    stop=True,
                )
                ob = obp.tile([128, 512], F32, tag="ob", name="ob")
                if final and qs % 2 == 1:
                    nc.scalar.activation(ob[:], pj[:], CPY)
                else:
                    nc.vector.tensor_copy(ob[:], pj[:])
                nc.sync.dma_start(out=out[i * 128 : (i + 1) * 128, :], in_=ob[:])

            def step_on_dve(qc, kt):
                # Deferred steps (kt < DEFER) must stay on ScalarE: their exp
                # is emitted before the PV-bank drain on the DVE, while their
                # PV is emitted after it on the PE — a DVE-exp there can
                # deadlock against the drain under scheduler reordering.
                if not use_dve_exp or qc == 0 or kt < 4:
                    return False
                return (kt - 4) % DVE_EXP_EVERY == 1

            with tc.tile_pool(name="ptp", bufs=10) as ptp, tc.tile_pool(
                name="smp", bufs=2
            ) as smp, tc.tile_pool(name="otp", bufs=2) as otp, tc.tile_pool(
                name="obp", bufs=2
            ) as obp, tc.tile_pool(
                name="psS", bufs=2, space="PSUM"
            ) as psS, tc.tile_pool(name="psV", bufs=1, space="PSUM") as psV:
                pv0 = psV.tile([65, 512], F32, tag="pv0", name="pv0")
                pv1 = psV.tile([65, 512], F32, tag="pv1", name="pv1")
                prebaked = {}

                # ---- prologue: per 512-token slice produce kT/qT/v, with
                # qc=0's attention interleaved so ScalarE starts early
                with tc.tile_pool(name="psA", bufs=2, space="PSUM") as psA:
                    for s in range(n_s):
                        sl = slice(s * 512, (s + 1) * 512)
                        # one DMA fetches this slice's tokens for all 4 chunks
                        nc.sync.dma_start(
                            out=xall[:].rearrange("p (k t) -> p k t", k=4)[:, :, sl],
                            in_=xt.rearrange("(k p) t -> p k t", k=4)[:, :, sl],
                        )
                        for w_sb, dst in ((wk_sb, kT), (wq_sb, qT)):
                            ps = psA.tile([128, 512], F32, tag="work", name="ps_kq")
                            for kc in range(4):
                                nc.tensor.matmul(
                                    ps[:],
                                    w_sb[:, kc * 128 : (kc + 1) * 128],
                                    xslc(kc, sl),
                                    start=(kc == 0),
                                    stop=(kc == 3),
                                )
                            # ScalarE drains kT/qT: keeps the DVE queue clear
                            # for the v copies and uses Act's prologue slack
                            nc.scalar.activation(dst[s][:], ps[:], CPY)
                        # S+exp for this slice's 4 key tiles first: ScalarE
                        # streams exps while the PE still produces v below.
                        pts = [
                            attn_S(0, kt, psS, ptp)
                            for kt in range(4 * s, 4 * s + 4)
                        ]
                        # v natural: per 128-token block, accumulate over kc
                        vn = psA.tile([128, 512], F32, tag="work", name="vn")
                        for j in range(4):
                            tb = slice((4 * s + j) * 128, (4 * s + j + 1) * 128)
                            for kc in range(4):
                                nc.tensor.matmul(
                                    vn[:, j * 128 : (j + 1) * 128],
                                    xslc(kc, tb),
                                    wv_sb[:, kc * 128 : (kc + 1) * 128],
                                    start=(kc == 0),
                                    stop=(kc == 3),
                                )
                        for j in range(4):
                            t = 4 * s + j
                            nc.vector.tensor_copy(
                                v[t][:, 0:64], vn[:, j * 128 : j * 128 + 64]
                            )
                            nc.vector.tensor_copy(
                                v[t][:, 65:129], vn[:, j * 128 + 64 : (j + 1) * 128]
                            )
                            nc.gpsimd.memset(v[t][:, 64:65], 1.0)
                            nc.gpsimd.memset(v[t][:, 129:130], 1.0)
                        for i, kt in enumerate(range(4 * s, 4 * s + 4)):
                            attn_PV(kt, pts[i], pv0, pv1)
                        # pre-bake chunk 1's first steps into prologue slack:
                        # ScalarE picks up extra exps where it would idle; the
                        # parked pt tiles feed chunk 1's deferred-PV phase.
                        if s in (2, 3) and n_qc > 1:
                            for kt in (2 * (s - 2), 2 * (s - 2) + 1):
                                prebaked[(1, kt)] = attn_S(1, kt, psS, ptp)

                # ---- steady state: chunks 1..n_qc-1. Chunk qc-1's epilogue is
                # interleaved into chunk qc's kt loop: the first DEFER steps
                # emit only S+exp while the DVE drains the previous chunk's PV
                # accumulators; their PV matmuls are emitted after the drain so
                # the in-order PE never blocks.
                DEFER = 8
                # bc + pj share one 2-buffer pool (same tag): 2 PSUM banks
                # total, and consecutive projections ping-pong between banks.
                with tc.tile_pool(name="psX", bufs=2, space="PSUM") as psX:
                    psB = psP = psX
                    prev_pv = (pv0, pv1)
                    prev_qc = 0
                    outT = None
                    for qc in range(1, n_qc):
                        pv0 = psV.tile([65, 512], F32, tag="pv0", name="pv0")
                        pv1 = psV.tile([65, 512], F32, tag="pv1", name="pv1")
                        pts = [
                            prebaked.pop((qc, kt), None)
                            or attn_S(qc, kt, psS, ptp, on_dve=step_on_dve(qc, kt))
                            for kt in range(DEFER)
                        ]
                        drained = epi_drain(prev_pv[0], prev_pv[1], smp, otp)
                        for kt in range(DEFER):
                            attn_PV(kt, pts[kt], pv0, pv1)
                        # DVE-exp steps get their PV delayed ~2 steps so the
                        # in-order PE streams past the slower DVE exp. The
                        # stop flag goes on the last PV actually emitted.
                        pending = []
                        n_emitted = DEFER
                        for kt in range(DEFER, n_xt):
                            on_dve = step_on_dve(qc, kt)
                            pt = attn_S(qc, kt, psS, ptp, on_dve=on_dve)
                            if on_dve:
                                pending.append((kt, pt))
                            else:
                                n_emitted += 1
                                last = n_emitted == n_xt
                                attn_PV(kt, pt, pv0, pv1, stop=last)
                                if pending and pending[0][0] <= kt - 2:
                                    dkt, dpt = pending.pop(0)
                                    n_emitted += 1
                                    attn_PV(
                                        dkt, dpt, pv0, pv1, stop=n_emitted == n_xt
                                    )
                            if kt == min(DEFER + 1, 9):
                                outT = epi_scale(*drained, smp, otp, psB)
                            if kt in (9, 12, 15, 18):
                                proj_qtile(prev_qc, (kt - 9) // 3, outT, psP, obp)
                        for dkt, dpt in pending:
                            n_emitted += 1
                            attn_PV(dkt, dpt, pv0, pv1, stop=n_emitted == n_xt)
                        prev_pv = (pv0, pv1)
                        prev_qc = qc
                    drained = epi_drain(prev_pv[0], prev_pv[1], smp, otp, final=True)
                    outT = epi_scale(*drained, smp, otp, psB)
                    for qs in range(4):
                        proj_qtile(prev_qc, qs, outT, psP, obp, final=True)
    nc.compile()
    return nc


_CACHE = {}


def _get_nc(tokens=N):
    if tokens not in _CACHE:
        _CACHE[tokens] = build(tokens)
    return _CACHE[tokens]


def _prep_w(w_slice):
    """[512, 128] -> [128, 512] fp16 with w_[p, kc*128 + j] = w[kc*128 + p, j]."""
    w = np.asarray(w_slice, dtype=np.float32)
    return np.ascontiguousarray(
        w.reshape(4, 128, 128).transpose(1, 0, 2).reshape(128, 512).astype(np.float16)
    )


def _shard_inputs(x, w_qkv, w_proj):
    in_maps = []
    for c in range(8):
        b, hp = divmod(c, 4)
        o = 128 * hp
        wall = np.concatenate(
            [
                _prep_w(w_qkv[:, o : o + 128]),
                _prep_w(w_qkv[:, 512 + o : 512 + o + 128]),
                _prep_w(w_qkv[:, 1024 + o : 1024 + o + 128]),
                w_proj[o : o + 128, :].astype(np.float16),
            ],
            axis=1,
        )
        in_maps.append(
            {
                "xt": np.ascontiguousarray(x[b].T.astype(np.float16)),
                "wall": np.ascontiguousarray(wall),
            }
        )
    return in_maps


def run(x, w_qkv, w_proj, b_proj, trace=False, **kwargs):
    from concourse.bass_utils import run_bass_kernel_spmd

    nc = _get_nc()
    in_maps = _shard_inputs(np.asarray(x), np.asarray(w_qkv), np.asarray(w_proj))
    br = run_bass_kernel_spmd(nc, in_maps, list(range(8)), trace=trace, **kwargs)
    parts = [np.asarray(br.results[c]["out"]) for c in range(8)]
    bp = np.asarray(b_proj)
    o0 = parts[0] + parts[1] + parts[2] + parts[3] + bp
    o1 = parts[4] + parts[5] + parts[6] + parts[7] + bp
    return np.stack([o0, o1]).astype(np.float32), br


def kernel(x, w_qkv, w_proj, b_proj):
    result, _ = run(x, w_qkv, w_proj, b_proj, trace=False)
    return result

